# revision 34
# baseline (speedup 1.0000x reference)
"""Trainium2 Bass kernel for the ContrastiveLearningModule loss.

Math (mirrors the reference):
  P = l2norm(relu(E @ W1.T + b1) @ W2.T + b2)  rowwise over [T,V,L,N,D]
  for each node type t, anchors idx[t][v,l,:]:
    pos  = sum_{(x,y) != (v,l)} exp(z . P[t,x,y,id]/TEMP)
    negi = sum_{s' != s}        exp(z . z_{s'}   /TEMP)
    negc = sum_{o,k}            exp(z . P[o,v,l,nid]/TEMP)
    loss = log(pos+negi+negc) - log(pos);  out = sum(loss)/1440

Sharding (no padding, all 8 cores run the identical program shape):
  24 (t,v,l) anchor groups -> per core: 1 full prio group (A: 100 anchors),
  1 half prio group (B: 50), 1 full rest group (C: 20), 1 half rest (D: 10)
  = exactly 1500 real projected columns per core, laid out as
  [anchors(+partners) 240 | pos 5xSa blocks 900 | negatives 360].

Device pipeline (per core), software-pipelined over 3 column chunks
(512/512/476) with a front(L1) / back(L2) / norm stage skew:
  PE: L1/L2 as fp8e4m3 DoubleRow matmuls (both 128-row contraction tiles
      in one pass; host pre-scales x by 8, W1 by 8, W2 by 16 so fp8
      operands sit in the normal range; the scales divide back out in the
      drains' scale arguments).  psn = ones@sq (bf16).  Sims matmuls
      (bf16 ph) write a slot-major merged PSUM pair (E tile: in-type
      windows + per-(slot,xy) pos dot columns via [128,Sa]x[128,1]
      matmuls; L tile: cross-type negatives), so den falls out of ONE
      contiguous row-reduce per slot per tile -- no transposes, no
      [1,900] row exps.
  ACT: relu-j0 / z-j0 drains, ln/exp 1/||z|| chain, one exp per merged
      sims tile, final Ln(den-e2)/Ln(pos).
  DVE: relu-j1 / z-j1 drains, sq, ph = z*rn, pos products for A/B,
      den = redE+redL, lossv.
  Pool: pos products for C/D, all den/pos row-reduces (off DVE).
  Per-anchor losses [100,4] DMA back; the host does the final masked sum.
  The first body also emits ~40 dummy matmuls during the input-DMA wait
  so the PE p-state ramp (0.65->2.4GHz over 3us) completes before L1.
"""

import sys

import numpy as np

sys.path.insert(0, "/opt/trn_rl_repo")

import concourse.bacc as bacc
import concourse.bass as bass
import concourse.mybir as mybir
import concourse.tile as tile
from concourse.bass_utils import run_bass_kernel_spmd  # noqa: F401 (contract)
from concourse.hw_specs import get_activation_tables as _real_gat

_ONE_TABLE = "natural_log_exp_and_others"  # relu/identity/exp/ln/square


def _gat_one_table(arch):
    """Pin the act-table chooser to a single set covering every ACT func we
    use so exactly one LoadActFuncSet is emitted."""
    tabs = _real_gat(arch)
    if _ONE_TABLE in tabs:
        return {k: (v if k == _ONE_TABLE else set()) for k, v in tabs.items()}
    return tabs


bacc.get_activation_tables = _gat_one_table

F32 = mybir.dt.float32
BF16 = mybir.dt.bfloat16
F8 = mybir.dt.float8e4
NP_F8 = mybir.dt.np(F8)
AF = mybir.ActivationFunctionType
ALU = mybir.AluOpType
DR = mybir.MatmulPerfMode.DoubleRow
NP_BF16 = mybir.dt.np(BF16)

# Problem constants (hardcoded per harness contract).
T, V, L, N, D = 4, 2, 3, 4000, 256
TEMP = 0.5
XY = V * L                      # 6 (view, layer) slabs
NCORES = 8
COUNT = 1440.0                  # total anchors in the reference loss
E2 = float(np.exp(2.0))         # self-similarity term exp(sim(z,z)/TEMP)
NC = 1500                       # packed columns per core
# column layout: [anchors 240 | pos 900 | negs 360]; the negs go last so
# the post-projection tail after the final chunk is only the (short)
# negative-sims chain.
CHUNKS = [(0, 512), (512, 1024), (1024, 1500)]

# fp8 pre-scales: x*8, W1*8 -> psh = 64*(W1x); h = relu(psh + 64*b1) = 64h
# (max |h| ~ 5 -> 320 < 448 fp8e4m3 limit); W2*16 -> psz = 1024*z.
XS = 8.0
W1S = 8.0
W2S = 16.0
HS = XS * W1S                   # h tile carries HS*h
ZS = HS * W2S                   # psz carries ZS*z

# slots: (name, anch, Sa, ewin, ewl, pos, negs, nwl, prb, eo, lo)
#   anch: anchor col base; ewin/ewl: in-type window (anchors+partners)
#   pos: 5*Sa pos block base; negs/nwl: cross-negatives window
#   prb: offset of slot's pos-product block inside pr (per jt)
#   eo: slot's base col in the E sims tile (ewl window cols then 5 pos
#   dot cols, contiguous per slot); lo: base col in the L sims tile.
SLOT_A = ("A", 0, 100, 0, 100, 640, 1140, 150, 400, 0, 0)
SLOT_B = ("B", 100, 50, 100, 100, 240, 1290, 150, 0, 100, 155)
SLOT_C = ("C", 200, 20, 200, 20, 490, 1440, 30, 250, 205, 305)
SLOT_D = ("D", 220, 10, 220, 20, 590, 1470, 30, 350, 230, 335)
SLOTS = [SLOT_A, SLOT_B, SLOT_C, SLOT_D]
PRW = 900                       # pos columns per jt (500+250+100+50)
EW = 256                        # E sims tile width (255 used)
LW = 368                        # L sims tile width (365 used)
PAL = 150                       # A pos dot col base inside the L tile

_CACHE = {}


def _emit_body(nc, tc, consts, dram, pools, rep, emit_w2=False):
    tail = _emit_proj(nc, tc, consts, dram, pools, rep, emit_w2)
    tail()


def _emit_proj(nc, tc, consts, dram, pools, rep, emit_w2=False):
    wt, bbt, ones128, epsb, ne2b = consts
    xt_d, wp_d, out_d = dram
    (xpool, hpool, zpool, sqpool, lnpool, rnpool, phpool, prpool,
     escpool, smallpool, psL, psM, psS) = pools
    r = f"r{rep}"

    # fp8 DoubleRow weight pairs: w1 for jout j = wt[:, 2j*128:(2j+2)*128]
    # viewed [128, 2(cin), 128]; w2 at +512.
    def wblk(layer, jout):
        o = layer * 512 + 2 * jout * 128
        return wt[:, o:o + 256].rearrange("p (two m) -> p two m", two=2)

    z = zpool.tile([128, 2 * NC], BF16, name=f"z{r}", tag="z")
    rn = rnpool.tile([128, 1536], BF16, name=f"rn{r}", tag="rn")
    ph = phpool.tile([128, 2 * NC], BF16, name=f"ph{r}", tag="ph")
    pr = prpool.tile([128, 2 * PRW], BF16, name=f"pr{r}", tag="pr")
    den1 = smallpool.tile([100, 4], F32, name=f"den1{r}", tag="den1")
    den2 = smallpool.tile([100, 4], F32, name=f"den2{r}", tag="den2")
    den = smallpool.tile([100, 4], F32, name=f"den{r}", tag="den")
    poss = smallpool.tile([100, 4], F32, name=f"poss{r}", tag="poss")
    lnden = smallpool.tile([100, 4], F32, name=f"lnden{r}", tag="lnden")
    lnpos = smallpool.tile([100, 4], F32, name=f"lnpos{r}", tag="lnpos")
    lossv = smallpool.tile([100, 4], F32, name=f"lossv{r}", tag="lossv")

    simps = {}

    def get_sims(key):
        if key not in simps:
            w = LW if key == "L" else EW
            simps[key] = psS.tile([100, w], F32, name=f"sim{key}{r}",
                                  tag=f"sim{key}",
                                  bufs=(1 if key == "L" else 2))
        return simps[key]

    def emit_sims_mm(si, late):
        """In-type / cross-type sims matmul pair (jt0+jt1 accum)."""
        nm, anch, Sa, ewin, ewl, _, negs, nwl, _, eo, lo = SLOTS[si]
        base, wl = (negs, nwl) if late else (ewin, ewl)
        sims = get_sims("L" if late else "E")
        off = lo if late else eo
        nc.tensor.matmul(sims[0:Sa, off:off + wl], ph[:, anch:anch + Sa],
                         ph[:, base:base + wl], start=True, stop=False)
        nc.tensor.matmul(sims[0:Sa, off:off + wl],
                         ph[:, NC + anch:NC + anch + Sa],
                         ph[:, NC + base:NC + base + wl], start=False,
                         stop=True)

    def emit_pr(si):
        """Pos products ph_pos * broadcast(anchor) for both jt.
        A/B on DVE (2x bf16), C/D on Pool."""
        nm, anch, Sa, _, _, pos, _, _, prb, _, _ = SLOTS[si]
        eng = nc.vector if si < 2 else nc.gpsimd
        for jt in (0, 1):
            zb = ph[:, jt * NC + anch:jt * NC + anch + Sa] \
                .unsqueeze(1).to_broadcast([128, 5, Sa])
            eng.tensor_mul(
                pr[:, jt * PRW + prb:jt * PRW + prb + 5 * Sa]
                .rearrange("p (a b) -> p a b", a=5),
                ph[:, jt * NC + pos:jt * NC + pos + 5 * Sa]
                .rearrange("p (a b) -> p a b", a=5),
                zb)

    def emit_pos_mm(si):
        """Per-(slot,xy) pos dot columns: [128,Sa] pr slice (stationary) x
        ones [128,1] -> sims[0:Sa, po+xy], accumulating jt0+jt1.  Slot A's
        dots land in the L tile (its products need the last chunk), the
        rest in the E tile."""
        nm, anch, Sa, _, ewl, _, _, _, prb, eo, _ = SLOTS[si]
        sims = get_sims("E" if si else "L")
        po = (eo + ewl) if si else PAL
        for xy in range(5):
            for jt in (0, 1):
                nc.tensor.matmul(
                    sims[0:Sa, po + xy:po + xy + 1],
                    pr[:, jt * PRW + prb + xy * Sa:
                       jt * PRW + prb + (xy + 1) * Sa],
                    ones128[:, 0:1], start=(jt == 0), stop=(jt == 1))

    def emit_escE():
        # E-side exps + reduces run inside proj: simE's last writer is
        # pos-mm C/D (chunk-1 products), so this frees simE a full tail
        # earlier and the den1/poss reduces overlap the last chunk.
        escE = escpool.tile([100, EW], BF16, name=f"escE{r}", tag="escE")
        nc.scalar.activation(escE[:, 0:255], simps["E"][:, 0:255], AF.Exp,
                             scale=2.0)
        for si, sl in enumerate(SLOTS):
            Sa, ewl, eo = sl[2], sl[4], sl[9]
            nc.vector.reduce_sum(den1[0:Sa, si:si + 1],
                                 escE[0:Sa, eo:eo + ewl + (5 if si else 0)],
                                 axis=mybir.AxisListType.X)
            if si:
                nc.vector.reduce_sum(poss[0:Sa, si:si + 1],
                                     escE[0:Sa, eo + ewl:eo + ewl + 5],
                                     axis=mybir.AxisListType.X)

    # ---- software-pipelined projection over 3 chunks ----
    NCH = len(CHUNKS)

    def emit_front(ci, st):
        lo, hi = CHUNKS[ci]
        cw = hi - lo
        xs = st["xs"]
        psh = [psL.tile([128, 512], F32, name=f"psh{ci}j{j}{r}", tag="l1")
               for j in (0, 1)]
        x2 = xs[:, 2 * lo:2 * lo + 2 * cw] \
            .rearrange("p (two c) -> p two c", two=2)
        for jout in (0, 1):
            nc.tensor.matmul(psh[jout][:, 0:cw], wblk(0, jout), x2,
                             start=True, stop=True, perf_mode=DR)
        h = hpool.tile([128, 1024], F8, name=f"h{ci}{r}", tag="h")
        nc.scalar.activation(h[:, 0:cw], psh[0][:, 0:cw], AF.Relu,
                             bias=bbt[:, 0:1])
        nc.vector.tensor_scalar(h[:, cw:2 * cw], psh[1][:, 0:cw],
                                bbt[:, 1:2], 0.0, ALU.add, ALU.max)
        st["h"][ci] = h

    def emit_back(ci, st):
        lo, hi = CHUNKS[ci]
        cw = hi - lo
        h = st["h"][ci]
        h2 = h[:, 0:2 * cw].rearrange("p (two c) -> p two c", two=2)
        psz = [psM.tile([128, 512], F32, name=f"psz{ci}j{j}{r}", tag="l2")
               for j in (0, 1)]
        for jout in (0, 1):
            nc.tensor.matmul(psz[jout][:, 0:cw], wblk(1, jout), h2,
                             start=True, stop=True, perf_mode=DR)
        # drains: z = psz/ZS + b2.  j0 on ACT; j1 on ACT too for chunks
        # 0/1 (pipeline slack hides it there), DVE for the last chunk
        # whose norm chain needs the two halves drained in parallel.
        nc.scalar.activation(z[:, lo:lo + cw], psz[0][:, 0:cw],
                             AF.Identity, bias=bbt[:, 2:3], scale=1.0 / ZS)
        if ci == 0:
            nc.scalar.activation(z[:, NC + lo:NC + lo + cw],
                                 psz[1][:, 0:cw], AF.Identity,
                                 bias=bbt[:, 3:4], scale=1.0 / ZS)
        else:
            nc.vector.tensor_scalar(z[:, NC + lo:NC + lo + cw],
                                    psz[1][:, 0:cw], 1.0 / ZS, bbt[:, 3:4],
                                    ALU.mult, ALU.add)

    def emit_norm(ci):
        lo, hi = CHUNKS[ci]
        cw = hi - lo
        zv = z[:].rearrange("p (j c) -> p j c", j=2)[:, :, lo:lo + cw]
        sq = sqpool.tile([128, 1024], BF16, name=f"sq{ci}{r}", tag="sq")
        nc.vector.tensor_mul(
            sq[:, 0:2 * cw].rearrange("p (j c) -> p j c", j=2), zv, zv)
        psn = psL.tile([128, 512], F32, name=f"pn{ci}{r}", tag="l1")
        nc.tensor.matmul(psn[:, 0:cw], ones128[:], sq[:, 0:cw],
                         start=True, stop=False)
        nc.tensor.matmul(psn[:, 0:cw], ones128[:], sq[:, cw:2 * cw],
                         start=False, stop=True)
        lnn = lnpool.tile([128, 512], F32, name=f"ln{ci}{r}", tag="lnn")
        nc.scalar.activation(lnn[:, 0:cw], psn[:, 0:cw], AF.Ln, bias=epsb[:])
        nc.scalar.activation(rn[:, lo:lo + cw], lnn[:, 0:cw], AF.Exp,
                             scale=-0.5)
        rnb = rn[:, lo:lo + cw].unsqueeze(1).to_broadcast([128, 2, cw])
        nc.vector.tensor_mul(
            ph[:].rearrange("p (j c) -> p j c", j=2)[:, :, lo:lo + cw],
            zv, rnb)

    st = {"h": {}}
    # one DMA per body: the chunk-major dram layout lands in a single tile
    xsall = xpool.tile([128, 2 * NC], F8, name=f"xs{r}", tag="xs")
    nc.sync.dma_start(xsall[:], xt_d[:])
    st["xs"] = xsall
    if emit_w2:
        # second weight half ordered after the x chunks so chunk-0 compute
        # starts as early as possible; w2 is not needed until L2-c0.
        nc.sync.dma_start(wt[:, 512:1024], wp_d[:, 512:1024])
        # PE p-state warmup: dummy matmuls filling the first-body DMA wait
        # so projection starts at full clock (2.4GHz needs 3us busy ramp).
        warm = psS.tile([100, EW], F32, name=f"warm{r}", tag="simE")
        for i in range(40):
            nc.tensor.matmul(warm[0:100, 0:128], ones128[:, 0:100],
                             ones128[:], start=True, stop=True)
    for t in range(NCH + 2):
        if t < NCH:
            emit_front(t, st)
        if 0 <= t - 1 < NCH:
            emit_back(t - 1, st)
        if 0 <= t - 2 < NCH:
            emit_norm(t - 2)
            if t - 2 == 0:
                # anchors all live in chunk 0: all in-type sims now
                for si in range(4):
                    emit_sims_mm(si, late=False)
                emit_pr(1)                # B pos inside chunk 0
                emit_pos_mm(1)
            elif t - 2 == 1:
                emit_pr(2)                # C pos spans chunks 0-1 (Pool)
                emit_pr(3)                # D pos inside chunk 1 (Pool)
                emit_pos_mm(2)
                emit_pos_mm(3)
            elif t - 2 == 2:
                emit_escE()

    def emit_tail():
        # ---- after the negs chunk: late sims + pos A + final loss ----
        emit_pr(0)                        # A pos spans chunks 1-2 (DVE)
        for si in range(4):
            emit_sims_mm(si, late=True)
        emit_pos_mm(0)
        escL = escpool.tile([100, LW], BF16, name=f"escL{r}", tag="escL")
        nc.scalar.activation(escL[:, 0:365], simps["L"][:, 0:365], AF.Exp,
                             scale=2.0)
        for si, sl in enumerate(SLOTS):
            Sa, lo = sl[2], sl[10]
            w = sl[7] + (5 if si == 0 else 0)
            nc.vector.reduce_sum(den2[0:Sa, si:si + 1],
                                 escL[0:Sa, lo:lo + w],
                                 axis=mybir.AxisListType.X)
        nc.vector.reduce_sum(poss[0:100, 0:1], escL[0:100, PAL:PAL + 5],
                             axis=mybir.AxisListType.X)
        nc.vector.tensor_add(den[:], den1[:], den2[:])
        nc.scalar.activation(lnden[:], den[:], AF.Ln, bias=ne2b[0:100, :])
        nc.scalar.activation(lnpos[:], poss[:], AF.Ln)
        nc.vector.tensor_sub(lossv[:], lnden[:], lnpos[:])
        # per-anchor losses go back to the host, which does the final
        # masked reduction (junk rows beyond each slot's Sa are not
        # summed).
        nc.sync.dma_start(out_d[:], lossv[:])
    return emit_tail


def _build_nc(reps=1, loop_iters=None, unroll=1, staggered=False):
    nc = bacc.Bacc("TRN2", target_bir_lowering=False, debug=False)

    xt_d = nc.dram_tensor("xt", [128, 2 * NC], F8, kind="ExternalInput")
    wp_d = nc.dram_tensor("wp", [128, 1024], F8, kind="ExternalInput")
    bb_d = nc.dram_tensor("bb", [128, 4], F32, kind="ExternalInput")
    out_d = nc.dram_tensor("out", [100, 4], F32, kind="ExternalOutput")

    with tile.TileContext(nc) as tc:
        with tc.tile_pool(name="const", bufs=1) as cpool:
            bbt = cpool.tile([128, 4], F32, name="bb", tag="bb")
            nc.sync.dma_start(bbt[:], bb_d[:])
            wt = cpool.tile([128, 1024], F8, name="wt", tag="wt")
            nc.sync.dma_start(wt[:, 0:512], wp_d[:, 0:512])
            ones128 = cpool.tile([128, 128], BF16, name="o128", tag="o128")
            nc.gpsimd.memset(ones128[:], 1.0)
            epsb = cpool.tile([128, 1], F32, name="epsb", tag="epsb")
            nc.gpsimd.memset(epsb[:], 1e-24)
            ne2b = cpool.tile([128, 1], F32, name="ne2b", tag="ne2b")
            nc.gpsimd.memset(ne2b[:], -E2)

            consts = (wt, bbt, ones128, epsb, ne2b)
            dram = (xt_d, wp_d, out_d)
            with (
                tc.tile_pool(name="xin", bufs=3) as xpool,
                tc.tile_pool(name="hbuf", bufs=3) as hpool,
                tc.tile_pool(name="zbuf", bufs=3) as zpool,
                tc.tile_pool(name="sqbuf", bufs=3) as sqpool,
                tc.tile_pool(name="lnbuf", bufs=3) as lnpool,
                tc.tile_pool(name="rnbuf", bufs=3) as rnpool,
                tc.tile_pool(name="phbuf", bufs=3) as phpool,
                tc.tile_pool(name="prbuf", bufs=3) as prpool,
                tc.tile_pool(name="escb", bufs=3) as escpool,
                tc.tile_pool(name="small", bufs=3) as smallpool,
                tc.tile_pool(name="psL", bufs=3,
                             space=bass.MemorySpace.PSUM) as psL,
                tc.tile_pool(name="psM", bufs=2,
                             space=bass.MemorySpace.PSUM) as psM,
                tc.tile_pool(name="psS", bufs=2,
                             space=bass.MemorySpace.PSUM) as psS,
            ):
                pools = (xpool, hpool, zpool, sqpool, lnpool, rnpool,
                         phpool, prpool, escpool, smallpool, psL, psM, psS)

                if loop_iters is not None:
                    nc.sync.dma_start(wt[:, 512:1024], wp_d[:, 512:1024])
                    with tc.For_i(0, loop_iters, 1,
                                  staggered_reset=staggered,
                                  hint_engines=(mybir.EngineType.PE,
                                                mybir.EngineType.DVE,
                                                mybir.EngineType.Activation)):
                        prev = None
                        for u in range(unroll):
                            cur = _emit_proj(nc, tc, consts, dram, pools, u)
                            if prev is not None:
                                prev()
                            prev = cur
                        prev()
                else:
                    prev = None
                    for rep in range(reps):
                        cur = _emit_proj(nc, tc, consts, dram, pools, rep,
                                         emit_w2=(rep == 0))
                        if prev is not None:
                            prev()
                        prev = cur
                    prev()

    nc.compile()
    return nc


def _get_nc(reps=1, loop_iters=None, unroll=1, staggered=False):
    key = ("nc", reps, loop_iters, unroll, staggered)
    if key not in _CACHE:
        _CACHE[key] = _build_nc(reps, loop_iters, unroll, staggered)
    return _CACHE[key]


def _assignments():
    """Per-core (A_full_prio, (B_group, half), C_full_rest, (D_group, half))."""
    P = [(t, v, l) for t in (0, 1) for v in range(V) for l in range(L)]
    R = [(t, v, l) for t in (2, 3) for v in range(V) for l in range(L)]
    out = []
    for c in range(NCORES):
        out.append((P[c], (P[8 + c // 2], c % 2), R[c], (R[8 + c // 2], c % 2)))
    return out


def make_in_maps(node_embeddings, W1, b1, W2, b2, idx_prio, idx_rest,
                 neg_idx_prio, neg_idx_rest):
    E = np.asarray(node_embeddings, dtype=np.float32)
    W1 = np.asarray(W1, dtype=np.float32)
    b1 = np.asarray(b1, dtype=np.float32)
    W2 = np.asarray(W2, dtype=np.float32)
    b2 = np.asarray(b2, dtype=np.float32)
    idxp = np.asarray(idx_prio)
    idxr = np.asarray(idx_rest)
    nidxp = np.asarray(neg_idx_prio)
    nidxr = np.asarray(neg_idx_rest)

    # weight blocks: [w1(j0,c0)|w1(j0,c1)|w1(j1,c0)|w1(j1,c1)|w2 same],
    # pre-scaled for fp8 (x*8, W1*8, W2*16).
    w1t, w2t = W1.T * W1S, W2.T * W2S
    blocks = []
    for wt_ in (w1t, w2t):
        for jout in (0, 1):
            for cin in (0, 1):
                blocks.append(wt_[cin * 128:(cin + 1) * 128,
                                  jout * 128:(jout + 1) * 128])
    wp = np.concatenate(blocks, axis=1).astype(NP_F8)
    bbm = np.stack([HS * b1[:128], HS * b1[128:], b2[:128], b2[128:]], axis=1)
    bbm = np.ascontiguousarray(bbm, dtype=np.float32)

    def others(t):
        return [u for u in range(T) if u != t]

    in_maps = []
    for (gA, (gB, hB), gC, (gD, hD)) in _assignments():
        # per-slot (t, v, l, own_ids, oth_ids, negs[3, K])
        slots = []
        t, v, l = gA
        ids = np.asarray(idxp[t][v, l])
        slots.append((t, v, l, ids, None, nidxp[t][v, l]))
        t, v, l = gB
        idf = np.asarray(idxp[t][v, l])
        slots.append((t, v, l, idf[hB * 50:hB * 50 + 50],
                      idf[(1 - hB) * 50:(1 - hB) * 50 + 50], nidxp[t][v, l]))
        t, v, l = gC
        ids = np.asarray(idxr[t - 2][v, l])
        slots.append((t, v, l, ids, None, nidxr[t - 2][v, l]))
        t, v, l = gD
        idf = np.asarray(idxr[t - 2][v, l])
        slots.append((t, v, l, idf[hD * 10:hD * 10 + 10],
                      idf[(1 - hD) * 10:(1 - hD) * 10 + 10],
                      nidxr[t - 2][v, l]))
        # layout: [anchors(+partners) 240 | pos B,C,D,A 900 | negs 360]
        # (pos block order B,C,D,A so B/C/D products are ready by chunk 1
        # and only A's pos products gate the tail)
        anch, posb, negs = [], [[], [], [], []], []
        for si, (t, v, l, own, oth, nvl) in enumerate(slots):
            anch.append(E[t, v, l, own])
            if oth is not None:
                anch.append(E[t, v, l, np.asarray(oth)])
            for x in range(V):
                for y in range(L):
                    if (x, y) != (v, l):
                        posb[si].append(E[t, x, y, own])
            for oi, u in enumerate(others(t)):
                negs.append(E[u, v, l, np.asarray(nvl[oi])])
        pos = posb[1] + posb[2] + posb[3] + posb[0]
        X = np.concatenate(anch + pos + negs, axis=0)   # [1500, 256]
        assert X.shape == (NC, D), X.shape
        XT = np.clip(X.T * XS, -448.0, 448.0).astype(NP_F8)  # [256, 1500]
        xt = np.empty((128, 2 * NC), NP_F8)
        for lo, hi in CHUNKS:
            cw = hi - lo
            for jt in (0, 1):
                xt[:, 2 * lo + jt * cw:2 * lo + (jt + 1) * cw] = \
                    XT[jt * 128:(jt + 1) * 128, lo:hi]
        in_maps.append({"xt": np.ascontiguousarray(xt), "wp": wp, "bb": bbm})
    return in_maps


def _make_runner(nc):
    """Lower nc to a cached jitted SPMD executable."""
    import jax
    from jax.experimental.shard_map import shard_map
    from jax.sharding import Mesh, PartitionSpec

    from concourse import bass2jax
    from concourse import mybir as mb

    bass2jax.install_neuronx_cc_hook()
    partition_name = (nc.partition_id_tensor.name
                      if nc.partition_id_tensor else None)
    in_names, out_names, out_avals = [], [], []
    for alloc in nc.m.functions[0].allocations:
        if not isinstance(alloc, mb.MemoryLocationSet):
            continue
        name = alloc.memorylocations[0].name
        if alloc.kind == "ExternalInput":
            if name != partition_name:
                in_names.append(name)
        elif alloc.kind == "ExternalOutput":
            out_names.append(name)
            out_avals.append(jax.core.ShapedArray(
                tuple(alloc.tensor_shape), mb.dt.np(alloc.dtype)))
    n_params = len(in_names)
    n_outs = len(out_avals)
    all_in_names = list(in_names) + list(out_names)
    if partition_name is not None:
        all_in_names.append(partition_name)

    def _body(*args):
        operands = list(args)
        if partition_name is not None:
            operands.append(bass2jax.partition_id_tensor())
        return tuple(bass2jax._bass_exec_p.bind(
            *operands,
            out_avals=tuple(out_avals),
            in_names=tuple(all_in_names),
            out_names=tuple(out_names),
            lowering_input_output_aliases=(),
            sim_require_finite=True,
            sim_require_nnan=True,
            nc=nc,
        ))

    devices = jax.devices()[:NCORES]
    mesh = Mesh(np.asarray(devices), ("core",))
    donate = tuple(range(n_params, n_params + n_outs))
    sharded = jax.jit(
        shard_map(_body, mesh=mesh,
                  in_specs=(PartitionSpec("core"),) * (n_params + n_outs),
                  out_specs=(PartitionSpec("core"),) * n_outs,
                  check_rep=False),
        donate_argnums=donate, keep_unused=True)

    def run(in_maps, device_inputs=None):
        if device_inputs is None:
            device_inputs = [
                np.concatenate([np.asarray(m[name]) for m in in_maps], axis=0)
                for name in in_names]
        zeros = [np.zeros((NCORES * a.shape[0], *a.shape[1:]), a.dtype)
                 for a in out_avals]
        out_arrs = sharded(*device_inputs, *zeros)
        return [
            {name: np.asarray(out_arrs[i]).reshape(NCORES, *out_avals[i].shape)[c]
             for i, name in enumerate(out_names)}
            for c in range(NCORES)
        ]

    run.in_names = in_names
    run.mesh = mesh
    return run


def _get_runner(reps=1, loop_iters=None, unroll=1, staggered=False):
    key = ("runner", reps, loop_iters, unroll, staggered)
    if key not in _CACHE:
        _CACHE[key] = _make_runner(
            _get_nc(reps, loop_iters, unroll, staggered))
    return _CACHE[key]


class _Res:
    def __init__(self, results):
        self.results = results


def run_on_hw(in_maps, reps=1, device_inputs=None, loop_iters=None,
              unroll=1, staggered=False):
    runner = _get_runner(reps, loop_iters, unroll, staggered)
    return _Res(runner(in_maps, device_inputs=device_inputs))


def kernel(node_embeddings, W1, b1, W2, b2, idx_prio, idx_rest,
           neg_idx_prio, neg_idx_rest, num_views=2, num_layers=3):
    in_maps = make_in_maps(node_embeddings, W1, b1, W2, b2, idx_prio,
                           idx_rest, neg_idx_prio, neg_idx_rest)
    res = run_on_hw(in_maps)
    _CACHE["last_results"] = res
    total = 0.0
    for c in range(NCORES):
        lv = np.asarray(res.results[c]["out"], dtype=np.float64)
        for si, sl in enumerate(SLOTS):
            total += lv[0:sl[2], si].sum()
    return np.float32(total / COUNT)


# revision 35
# speedup vs baseline: 1.3974x; 1.3974x over previous
"""Trainium2 Bass kernel for the ContrastiveLearningModule loss.

Math (mirrors the reference):
  P = l2norm(relu(E @ W1.T + b1) @ W2.T + b2)  rowwise over [T,V,L,N,D]
  for each node type t, anchors idx[t][v,l,:]:
    pos  = sum_{(x,y) != (v,l)} exp(z . P[t,x,y,id]/TEMP)
    negi = sum_{s' != s}        exp(z . z_{s'}   /TEMP)
    negc = sum_{o,k}            exp(z . P[o,v,l,nid]/TEMP)
    loss = log(pos+negi+negc) - log(pos);  out = sum(loss)/1440

Sharding (no padding, all 8 cores run the identical program shape):
  24 (t,v,l) anchor groups -> per core: 1 full prio group (A: 100 anchors),
  1 half prio group (B: 50), 1 full rest group (C: 20), 1 half rest (D: 10)
  = exactly 1500 real projected columns per core, laid out as
  [anchors(+partners) 240 | pos 5xSa blocks 900 | negatives 360].

Device pipeline (per core), software-pipelined over 3 column chunks
(512/512/476) with a front(L1) / back(L2) / norm stage skew:
  PE: L1/L2 as fp8e4m3 DoubleRow matmuls (both 128-row contraction tiles
      in one pass; host pre-scales x by 8, W1 by 8, W2 by 16 so fp8
      operands sit in the normal range; the scales divide back out in the
      drains' scale arguments).  psn = ones@sq (bf16).  Sims matmuls
      (bf16 ph) write a slot-major merged PSUM pair (E tile: in-type
      windows + per-(slot,xy) pos dot columns via [128,Sa]x[128,1]
      matmuls; L tile: cross-type negatives), so den falls out of ONE
      contiguous row-reduce per slot per tile -- no transposes, no
      [1,900] row exps.
  ACT: relu-j0 / z-j0 drains, ln/exp 1/||z|| chain, one exp per merged
      sims tile, final Ln(den-e2)/Ln(pos).
  DVE: relu-j1 / z-j1 drains, sq, ph = z*rn, pos products for A/B,
      den = redE+redL, lossv.
  Pool: pos products for C/D, all den/pos row-reduces (off DVE).
  Per-anchor losses [100,4] DMA back; the host does the final masked sum.
  The first body also emits ~40 dummy matmuls during the input-DMA wait
  so the PE p-state ramp (0.65->2.4GHz over 3us) completes before L1.
"""

import sys

import numpy as np

sys.path.insert(0, "/opt/trn_rl_repo")

import concourse.bacc as bacc
import concourse.bass as bass
import concourse.mybir as mybir
import concourse.tile as tile
from concourse.bass_utils import run_bass_kernel_spmd  # noqa: F401 (contract)
from concourse.hw_specs import get_activation_tables as _real_gat

_ONE_TABLE = "natural_log_exp_and_others"  # relu/identity/exp/ln/square


def _gat_one_table(arch):
    """Pin the act-table chooser to a single set covering every ACT func we
    use so exactly one LoadActFuncSet is emitted."""
    tabs = _real_gat(arch)
    if _ONE_TABLE in tabs:
        return {k: (v if k == _ONE_TABLE else set()) for k, v in tabs.items()}
    return tabs


bacc.get_activation_tables = _gat_one_table

F32 = mybir.dt.float32
BF16 = mybir.dt.bfloat16
F8 = mybir.dt.float8e4
NP_F8 = mybir.dt.np(F8)
AF = mybir.ActivationFunctionType
ALU = mybir.AluOpType
DR = mybir.MatmulPerfMode.DoubleRow
NP_BF16 = mybir.dt.np(BF16)

# Problem constants (hardcoded per harness contract).
T, V, L, N, D = 4, 2, 3, 4000, 256
TEMP = 0.5
XY = V * L                      # 6 (view, layer) slabs
NCORES = 8
COUNT = 1440.0                  # total anchors in the reference loss
E2 = float(np.exp(2.0))         # self-similarity term exp(sim(z,z)/TEMP)
NC = 1500                       # packed columns per core
# column layout: [anchors 240 | pos 900 | negs 360]; the negs go last so
# the post-projection tail after the final chunk is only the (short)
# negative-sims chain.
CHUNKS = [(0, 512), (512, 1024), (1024, 1500)]

# fp8 pre-scales: x*8, W1*8 -> psh = 64*(W1x); h = relu(psh + 64*b1) = 64h
# (max |h| ~ 5 -> 320 < 448 fp8e4m3 limit); W2*16 -> psz = 1024*z.
XS = 8.0
W1S = 8.0
W2S = 16.0
HS = XS * W1S                   # h tile carries HS*h
ZS = HS * W2S                   # psz carries ZS*z

# slots: (name, anch, Sa, ewin, ewl, pos, negs, nwl, prb, eo, lo)
#   anch: anchor col base; ewin/ewl: in-type window (anchors+partners)
#   pos: 5*Sa pos block base; negs/nwl: cross-negatives window
#   prb: offset of slot's pos-product block inside pr (per jt)
#   eo: slot's base col in the E sims tile (ewl window cols then 5 pos
#   dot cols, contiguous per slot); lo: base col in the L sims tile.
SLOT_A = ("A", 0, 100, 0, 100, 640, 1140, 150, 400, 0, 0)
SLOT_B = ("B", 100, 50, 100, 100, 240, 1290, 150, 0, 100, 155)
SLOT_C = ("C", 200, 20, 200, 20, 490, 1440, 30, 250, 205, 305)
SLOT_D = ("D", 220, 10, 220, 20, 590, 1470, 30, 350, 230, 335)
SLOTS = [SLOT_A, SLOT_B, SLOT_C, SLOT_D]
PRW = 900                       # pos columns per jt (500+250+100+50)
EW = 256                        # E sims tile width (255 used)
LW = 368                        # L sims tile width (365 used)
PAL = 150                       # A pos dot col base inside the L tile

_CACHE = {}


def _emit_body(nc, tc, consts, dram, pools, rep, emit_w2=False):
    tail = _emit_proj(nc, tc, consts, dram, pools, rep, emit_w2)
    tail()


def _emit_proj(nc, tc, consts, dram, pools, rep, emit_w2=False):
    wt, bbt, ones128, epsb, ne2b = consts
    xt_d, wp_d, out_d = dram
    (xpool, hpool, zpool, sqpool, lnpool, rnpool, phpool, prpool,
     escpool, smallpool, psL, psM, psS) = pools
    r = f"r{rep}"

    # fp8 DoubleRow weight pairs: w1 for jout j = wt[:, 2j*128:(2j+2)*128]
    # viewed [128, 2(cin), 128]; w2 at +512.
    def wblk(layer, jout):
        o = layer * 512 + 2 * jout * 128
        return wt[:, o:o + 256].rearrange("p (two m) -> p two m", two=2)

    z = zpool.tile([128, 2 * NC], BF16, name=f"z{r}", tag="z")
    rn = rnpool.tile([128, 1536], BF16, name=f"rn{r}", tag="rn")
    ph = phpool.tile([128, 2 * NC], BF16, name=f"ph{r}", tag="ph")
    pr = prpool.tile([128, 2 * PRW], BF16, name=f"pr{r}", tag="pr")
    den1 = smallpool.tile([100, 4], F32, name=f"den1{r}", tag="den1")
    den2 = smallpool.tile([100, 4], F32, name=f"den2{r}", tag="den2")
    den = smallpool.tile([100, 4], F32, name=f"den{r}", tag="den")
    poss = smallpool.tile([100, 4], F32, name=f"poss{r}", tag="poss")
    lnden = smallpool.tile([100, 4], F32, name=f"lnden{r}", tag="lnden")
    lnpos = smallpool.tile([100, 4], F32, name=f"lnpos{r}", tag="lnpos")
    lossv = smallpool.tile([100, 4], F32, name=f"lossv{r}", tag="lossv")

    simps = {}

    def get_sims(key):
        if key not in simps:
            w = LW if key == "L" else EW
            simps[key] = psS.tile([100, w], F32, name=f"sim{key}{r}",
                                  tag=f"sim{key}",
                                  bufs=(1 if key == "L" else 2))
        return simps[key]

    def emit_sims_mm(si, late):
        """In-type / cross-type sims matmul pair (jt0+jt1 accum)."""
        nm, anch, Sa, ewin, ewl, _, negs, nwl, _, eo, lo = SLOTS[si]
        base, wl = (negs, nwl) if late else (ewin, ewl)
        sims = get_sims("L" if late else "E")
        off = lo if late else eo
        nc.tensor.matmul(sims[0:Sa, off:off + wl], ph[:, anch:anch + Sa],
                         ph[:, base:base + wl], start=True, stop=False)
        nc.tensor.matmul(sims[0:Sa, off:off + wl],
                         ph[:, NC + anch:NC + anch + Sa],
                         ph[:, NC + base:NC + base + wl], start=False,
                         stop=True)

    def emit_pr(si):
        """Pos products ph_pos * broadcast(anchor) for both jt.
        A/B on DVE (2x bf16), C/D on Pool."""
        nm, anch, Sa, _, _, pos, _, _, prb, _, _ = SLOTS[si]
        eng = nc.vector if si < 2 else nc.gpsimd
        for jt in (0, 1):
            zb = ph[:, jt * NC + anch:jt * NC + anch + Sa] \
                .unsqueeze(1).to_broadcast([128, 5, Sa])
            eng.tensor_mul(
                pr[:, jt * PRW + prb:jt * PRW + prb + 5 * Sa]
                .rearrange("p (a b) -> p a b", a=5),
                ph[:, jt * NC + pos:jt * NC + pos + 5 * Sa]
                .rearrange("p (a b) -> p a b", a=5),
                zb)

    def emit_pos_mm(si):
        """Per-(slot,xy) pos dot columns: [128,Sa] pr slice (stationary) x
        ones [128,1] -> sims[0:Sa, po+xy], accumulating jt0+jt1.  Slot A's
        dots land in the L tile (its products need the last chunk), the
        rest in the E tile."""
        nm, anch, Sa, _, ewl, _, _, _, prb, eo, _ = SLOTS[si]
        sims = get_sims("E" if si else "L")
        po = (eo + ewl) if si else PAL
        for xy in range(5):
            for jt in (0, 1):
                nc.tensor.matmul(
                    sims[0:Sa, po + xy:po + xy + 1],
                    pr[:, jt * PRW + prb + xy * Sa:
                       jt * PRW + prb + (xy + 1) * Sa],
                    ones128[:, 0:1], start=(jt == 0), stop=(jt == 1))

    def emit_escE():
        # E-side exps + reduces run inside proj: simE's last writer is
        # pos-mm C/D (chunk-1 products), so this frees simE a full tail
        # earlier and the den1/poss reduces overlap the last chunk.
        escE = escpool.tile([100, EW], BF16, name=f"escE{r}", tag="escE")
        nc.scalar.activation(escE[:, 0:255], simps["E"][:, 0:255], AF.Exp,
                             scale=2.0)
        for si, sl in enumerate(SLOTS):
            Sa, ewl, eo = sl[2], sl[4], sl[9]
            nc.vector.reduce_sum(den1[0:Sa, si:si + 1],
                                 escE[0:Sa, eo:eo + ewl + (5 if si else 0)],
                                 axis=mybir.AxisListType.X)
            if si:
                nc.vector.reduce_sum(poss[0:Sa, si:si + 1],
                                     escE[0:Sa, eo + ewl:eo + ewl + 5],
                                     axis=mybir.AxisListType.X)

    # ---- software-pipelined projection over 3 chunks ----
    NCH = len(CHUNKS)

    def emit_front(ci, st):
        lo, hi = CHUNKS[ci]
        cw = hi - lo
        xs = st["xs"]
        psh = [psL.tile([128, 512], F32, name=f"psh{ci}j{j}{r}", tag="l1")
               for j in (0, 1)]
        x2 = xs[:, 2 * lo:2 * lo + 2 * cw] \
            .rearrange("p (two c) -> p two c", two=2)
        for jout in (0, 1):
            nc.tensor.matmul(psh[jout][:, 0:cw], wblk(0, jout), x2,
                             start=True, stop=True, perf_mode=DR)
        h = hpool.tile([128, 1024], F8, name=f"h{ci}{r}", tag="h")
        nc.scalar.activation(h[:, 0:cw], psh[0][:, 0:cw], AF.Relu,
                             bias=bbt[:, 0:1])
        nc.vector.tensor_scalar(h[:, cw:2 * cw], psh[1][:, 0:cw],
                                bbt[:, 1:2], 0.0, ALU.add, ALU.max)
        st["h"][ci] = h

    def emit_back(ci, st):
        lo, hi = CHUNKS[ci]
        cw = hi - lo
        h = st["h"][ci]
        h2 = h[:, 0:2 * cw].rearrange("p (two c) -> p two c", two=2)
        psz = [psM.tile([128, 512], F32, name=f"psz{ci}j{j}{r}", tag="l2")
               for j in (0, 1)]
        for jout in (0, 1):
            nc.tensor.matmul(psz[jout][:, 0:cw], wblk(1, jout), h2,
                             start=True, stop=True, perf_mode=DR)
        # drains: z = psz/ZS + b2.  j0 on ACT; j1 on ACT too for chunks
        # 0/1 (pipeline slack hides it there), DVE for the last chunk
        # whose norm chain needs the two halves drained in parallel.
        nc.scalar.activation(z[:, lo:lo + cw], psz[0][:, 0:cw],
                             AF.Identity, bias=bbt[:, 2:3], scale=1.0 / ZS)
        if ci == len(CHUNKS) - 1:
            nc.vector.tensor_scalar(z[:, NC + lo:NC + lo + cw],
                                    psz[1][:, 0:cw], 1.0 / ZS, bbt[:, 3:4],
                                    ALU.mult, ALU.add)
        else:
            nc.scalar.activation(z[:, NC + lo:NC + lo + cw],
                                 psz[1][:, 0:cw], AF.Identity,
                                 bias=bbt[:, 3:4], scale=1.0 / ZS)

    def emit_norm(ci):
        lo, hi = CHUNKS[ci]
        cw = hi - lo
        zv = z[:].rearrange("p (j c) -> p j c", j=2)[:, :, lo:lo + cw]
        sq = sqpool.tile([128, 1024], BF16, name=f"sq{ci}{r}", tag="sq")
        nc.vector.tensor_mul(
            sq[:, 0:2 * cw].rearrange("p (j c) -> p j c", j=2), zv, zv)
        psn = psL.tile([128, 512], F32, name=f"pn{ci}{r}", tag="l1")
        nc.tensor.matmul(psn[:, 0:cw], ones128[:], sq[:, 0:cw],
                         start=True, stop=False)
        nc.tensor.matmul(psn[:, 0:cw], ones128[:], sq[:, cw:2 * cw],
                         start=False, stop=True)
        lnn = lnpool.tile([128, 512], F32, name=f"ln{ci}{r}", tag="lnn")
        nc.scalar.activation(lnn[:, 0:cw], psn[:, 0:cw], AF.Ln, bias=epsb[:])
        nc.scalar.activation(rn[:, lo:lo + cw], lnn[:, 0:cw], AF.Exp,
                             scale=-0.5)
        rnb = rn[:, lo:lo + cw].unsqueeze(1).to_broadcast([128, 2, cw])
        nc.vector.tensor_mul(
            ph[:].rearrange("p (j c) -> p j c", j=2)[:, :, lo:lo + cw],
            zv, rnb)

    st = {"h": {}}
    # one DMA per body: the chunk-major dram layout lands in a single tile
    xsall = xpool.tile([128, 2 * NC], F8, name=f"xs{r}", tag="xs")
    nc.sync.dma_start(xsall[:], xt_d[:])
    st["xs"] = xsall
    if emit_w2:
        # second weight half ordered after the x chunks so chunk-0 compute
        # starts as early as possible; w2 is not needed until L2-c0.
        nc.sync.dma_start(wt[:, 512:1024], wp_d[:, 512:1024])
        # PE p-state warmup: dummy matmuls filling the first-body DMA wait
        # so projection starts at full clock (2.4GHz needs 3us busy ramp).
        warm = psS.tile([100, EW], F32, name=f"warm{r}", tag="simE")
        for i in range(40):
            nc.tensor.matmul(warm[0:100, 0:128], ones128[:, 0:100],
                             ones128[:], start=True, stop=True)
    for t in range(NCH + 2):
        if t < NCH:
            emit_front(t, st)
        if 0 <= t - 1 < NCH:
            emit_back(t - 1, st)
        if 0 <= t - 2 < NCH:
            emit_norm(t - 2)
            if t - 2 == 0:
                # anchors all live in chunk 0: all in-type sims now
                for si in range(4):
                    emit_sims_mm(si, late=False)
                emit_pr(1)                # B pos inside chunk 0
                emit_pos_mm(1)
            elif t - 2 == 1:
                emit_pr(2)                # C pos spans chunks 0-1 (Pool)
                emit_pr(3)                # D pos inside chunk 1 (Pool)
                emit_pos_mm(2)
                emit_pos_mm(3)
            elif t - 2 == 2:
                emit_escE()

    def emit_tail():
        # ---- after the negs chunk: late sims + pos A + final loss ----
        emit_pr(0)                        # A pos spans chunks 1-2 (DVE)
        for si in range(4):
            emit_sims_mm(si, late=True)
        emit_pos_mm(0)
        escL = escpool.tile([100, LW], BF16, name=f"escL{r}", tag="escL")
        nc.scalar.activation(escL[:, 0:365], simps["L"][:, 0:365], AF.Exp,
                             scale=2.0)
        for si, sl in enumerate(SLOTS):
            Sa, lo = sl[2], sl[10]
            w = sl[7] + (5 if si == 0 else 0)
            nc.vector.reduce_sum(den2[0:Sa, si:si + 1],
                                 escL[0:Sa, lo:lo + w],
                                 axis=mybir.AxisListType.X)
        nc.vector.reduce_sum(poss[0:100, 0:1], escL[0:100, PAL:PAL + 5],
                             axis=mybir.AxisListType.X)
        nc.vector.tensor_add(den[:], den1[:], den2[:])
        nc.scalar.activation(lnden[:], den[:], AF.Ln, bias=ne2b[0:100, :])
        nc.scalar.activation(lnpos[:], poss[:], AF.Ln)
        nc.vector.tensor_sub(lossv[:], lnden[:], lnpos[:])
        # per-anchor losses go back to the host, which does the final
        # masked reduction (junk rows beyond each slot's Sa are not
        # summed).
        nc.sync.dma_start(out_d[:], lossv[:])
    return emit_tail


def _build_nc(reps=1, loop_iters=None, unroll=1, staggered=False):
    nc = bacc.Bacc("TRN2", target_bir_lowering=False, debug=False)

    xt_d = nc.dram_tensor("xt", [128, 2 * NC], F8, kind="ExternalInput")
    wp_d = nc.dram_tensor("wp", [128, 1024], F8, kind="ExternalInput")
    bb_d = nc.dram_tensor("bb", [128, 4], F32, kind="ExternalInput")
    out_d = nc.dram_tensor("out", [100, 4], F32, kind="ExternalOutput")

    with tile.TileContext(nc) as tc:
        with tc.tile_pool(name="const", bufs=1) as cpool:
            bbt = cpool.tile([128, 4], F32, name="bb", tag="bb")
            nc.sync.dma_start(bbt[:], bb_d[:])
            wt = cpool.tile([128, 1024], F8, name="wt", tag="wt")
            nc.sync.dma_start(wt[:, 0:512], wp_d[:, 0:512])
            ones128 = cpool.tile([128, 128], BF16, name="o128", tag="o128")
            nc.gpsimd.memset(ones128[:], 1.0)
            epsb = cpool.tile([128, 1], F32, name="epsb", tag="epsb")
            nc.gpsimd.memset(epsb[:], 1e-24)
            ne2b = cpool.tile([128, 1], F32, name="ne2b", tag="ne2b")
            nc.gpsimd.memset(ne2b[:], -E2)

            consts = (wt, bbt, ones128, epsb, ne2b)
            dram = (xt_d, wp_d, out_d)
            with (
                tc.tile_pool(name="xin", bufs=3) as xpool,
                tc.tile_pool(name="hbuf", bufs=3) as hpool,
                tc.tile_pool(name="zbuf", bufs=3) as zpool,
                tc.tile_pool(name="sqbuf", bufs=3) as sqpool,
                tc.tile_pool(name="lnbuf", bufs=3) as lnpool,
                tc.tile_pool(name="rnbuf", bufs=3) as rnpool,
                tc.tile_pool(name="phbuf", bufs=3) as phpool,
                tc.tile_pool(name="prbuf", bufs=3) as prpool,
                tc.tile_pool(name="escb", bufs=3) as escpool,
                tc.tile_pool(name="small", bufs=3) as smallpool,
                tc.tile_pool(name="psL", bufs=3,
                             space=bass.MemorySpace.PSUM) as psL,
                tc.tile_pool(name="psM", bufs=2,
                             space=bass.MemorySpace.PSUM) as psM,
                tc.tile_pool(name="psS", bufs=2,
                             space=bass.MemorySpace.PSUM) as psS,
            ):
                pools = (xpool, hpool, zpool, sqpool, lnpool, rnpool,
                         phpool, prpool, escpool, smallpool, psL, psM, psS)

                if loop_iters is not None:
                    nc.sync.dma_start(wt[:, 512:1024], wp_d[:, 512:1024])
                    with tc.For_i(0, loop_iters, 1,
                                  staggered_reset=staggered,
                                  hint_engines=(mybir.EngineType.PE,
                                                mybir.EngineType.DVE,
                                                mybir.EngineType.Activation)):
                        prev = None
                        for u in range(unroll):
                            cur = _emit_proj(nc, tc, consts, dram, pools, u)
                            if prev is not None:
                                prev()
                            prev = cur
                        prev()
                else:
                    prev = None
                    for rep in range(reps):
                        cur = _emit_proj(nc, tc, consts, dram, pools, rep,
                                         emit_w2=(rep == 0))
                        if prev is not None:
                            prev()
                        prev = cur
                    prev()

    nc.compile()
    return nc


def _get_nc(reps=1, loop_iters=None, unroll=1, staggered=False):
    key = ("nc", reps, loop_iters, unroll, staggered)
    if key not in _CACHE:
        _CACHE[key] = _build_nc(reps, loop_iters, unroll, staggered)
    return _CACHE[key]


def _assignments():
    """Per-core (A_full_prio, (B_group, half), C_full_rest, (D_group, half))."""
    P = [(t, v, l) for t in (0, 1) for v in range(V) for l in range(L)]
    R = [(t, v, l) for t in (2, 3) for v in range(V) for l in range(L)]
    out = []
    for c in range(NCORES):
        out.append((P[c], (P[8 + c // 2], c % 2), R[c], (R[8 + c // 2], c % 2)))
    return out


def make_in_maps(node_embeddings, W1, b1, W2, b2, idx_prio, idx_rest,
                 neg_idx_prio, neg_idx_rest):
    E = np.asarray(node_embeddings, dtype=np.float32)
    W1 = np.asarray(W1, dtype=np.float32)
    b1 = np.asarray(b1, dtype=np.float32)
    W2 = np.asarray(W2, dtype=np.float32)
    b2 = np.asarray(b2, dtype=np.float32)
    idxp = np.asarray(idx_prio)
    idxr = np.asarray(idx_rest)
    nidxp = np.asarray(neg_idx_prio)
    nidxr = np.asarray(neg_idx_rest)

    # weight blocks: [w1(j0,c0)|w1(j0,c1)|w1(j1,c0)|w1(j1,c1)|w2 same],
    # pre-scaled for fp8 (x*8, W1*8, W2*16).
    w1t, w2t = W1.T * W1S, W2.T * W2S
    blocks = []
    for wt_ in (w1t, w2t):
        for jout in (0, 1):
            for cin in (0, 1):
                blocks.append(wt_[cin * 128:(cin + 1) * 128,
                                  jout * 128:(jout + 1) * 128])
    wp = np.concatenate(blocks, axis=1).astype(NP_F8)
    bbm = np.stack([HS * b1[:128], HS * b1[128:], b2[:128], b2[128:]], axis=1)
    bbm = np.ascontiguousarray(bbm, dtype=np.float32)

    def others(t):
        return [u for u in range(T) if u != t]

    in_maps = []
    for (gA, (gB, hB), gC, (gD, hD)) in _assignments():
        # per-slot (t, v, l, own_ids, oth_ids, negs[3, K])
        slots = []
        t, v, l = gA
        ids = np.asarray(idxp[t][v, l])
        slots.append((t, v, l, ids, None, nidxp[t][v, l]))
        t, v, l = gB
        idf = np.asarray(idxp[t][v, l])
        slots.append((t, v, l, idf[hB * 50:hB * 50 + 50],
                      idf[(1 - hB) * 50:(1 - hB) * 50 + 50], nidxp[t][v, l]))
        t, v, l = gC
        ids = np.asarray(idxr[t - 2][v, l])
        slots.append((t, v, l, ids, None, nidxr[t - 2][v, l]))
        t, v, l = gD
        idf = np.asarray(idxr[t - 2][v, l])
        slots.append((t, v, l, idf[hD * 10:hD * 10 + 10],
                      idf[(1 - hD) * 10:(1 - hD) * 10 + 10],
                      nidxr[t - 2][v, l]))
        # layout: [anchors(+partners) 240 | pos B,C,D,A 900 | negs 360]
        # (pos block order B,C,D,A so B/C/D products are ready by chunk 1
        # and only A's pos products gate the tail)
        anch, posb, negs = [], [[], [], [], []], []
        for si, (t, v, l, own, oth, nvl) in enumerate(slots):
            anch.append(E[t, v, l, own])
            if oth is not None:
                anch.append(E[t, v, l, np.asarray(oth)])
            for x in range(V):
                for y in range(L):
                    if (x, y) != (v, l):
                        posb[si].append(E[t, x, y, own])
            for oi, u in enumerate(others(t)):
                negs.append(E[u, v, l, np.asarray(nvl[oi])])
        pos = posb[1] + posb[2] + posb[3] + posb[0]
        X = np.concatenate(anch + pos + negs, axis=0)   # [1500, 256]
        assert X.shape == (NC, D), X.shape
        XT = np.clip(X.T * XS, -448.0, 448.0).astype(NP_F8)  # [256, 1500]
        xt = np.empty((128, 2 * NC), NP_F8)
        for lo, hi in CHUNKS:
            cw = hi - lo
            for jt in (0, 1):
                xt[:, 2 * lo + jt * cw:2 * lo + (jt + 1) * cw] = \
                    XT[jt * 128:(jt + 1) * 128, lo:hi]
        in_maps.append({"xt": np.ascontiguousarray(xt), "wp": wp, "bb": bbm})
    return in_maps


def _make_runner(nc):
    """Lower nc to a cached jitted SPMD executable."""
    import jax
    from jax.experimental.shard_map import shard_map
    from jax.sharding import Mesh, PartitionSpec

    from concourse import bass2jax
    from concourse import mybir as mb

    bass2jax.install_neuronx_cc_hook()
    partition_name = (nc.partition_id_tensor.name
                      if nc.partition_id_tensor else None)
    in_names, out_names, out_avals = [], [], []
    for alloc in nc.m.functions[0].allocations:
        if not isinstance(alloc, mb.MemoryLocationSet):
            continue
        name = alloc.memorylocations[0].name
        if alloc.kind == "ExternalInput":
            if name != partition_name:
                in_names.append(name)
        elif alloc.kind == "ExternalOutput":
            out_names.append(name)
            out_avals.append(jax.core.ShapedArray(
                tuple(alloc.tensor_shape), mb.dt.np(alloc.dtype)))
    n_params = len(in_names)
    n_outs = len(out_avals)
    all_in_names = list(in_names) + list(out_names)
    if partition_name is not None:
        all_in_names.append(partition_name)

    def _body(*args):
        operands = list(args)
        if partition_name is not None:
            operands.append(bass2jax.partition_id_tensor())
        return tuple(bass2jax._bass_exec_p.bind(
            *operands,
            out_avals=tuple(out_avals),
            in_names=tuple(all_in_names),
            out_names=tuple(out_names),
            lowering_input_output_aliases=(),
            sim_require_finite=True,
            sim_require_nnan=True,
            nc=nc,
        ))

    devices = jax.devices()[:NCORES]
    mesh = Mesh(np.asarray(devices), ("core",))
    donate = tuple(range(n_params, n_params + n_outs))
    sharded = jax.jit(
        shard_map(_body, mesh=mesh,
                  in_specs=(PartitionSpec("core"),) * (n_params + n_outs),
                  out_specs=(PartitionSpec("core"),) * n_outs,
                  check_rep=False),
        donate_argnums=donate, keep_unused=True)

    def run(in_maps, device_inputs=None):
        if device_inputs is None:
            device_inputs = [
                np.concatenate([np.asarray(m[name]) for m in in_maps], axis=0)
                for name in in_names]
        zeros = [np.zeros((NCORES * a.shape[0], *a.shape[1:]), a.dtype)
                 for a in out_avals]
        out_arrs = sharded(*device_inputs, *zeros)
        return [
            {name: np.asarray(out_arrs[i]).reshape(NCORES, *out_avals[i].shape)[c]
             for i, name in enumerate(out_names)}
            for c in range(NCORES)
        ]

    run.in_names = in_names
    run.mesh = mesh
    return run


def _get_runner(reps=1, loop_iters=None, unroll=1, staggered=False):
    key = ("runner", reps, loop_iters, unroll, staggered)
    if key not in _CACHE:
        _CACHE[key] = _make_runner(
            _get_nc(reps, loop_iters, unroll, staggered))
    return _CACHE[key]


class _Res:
    def __init__(self, results):
        self.results = results


def run_on_hw(in_maps, reps=1, device_inputs=None, loop_iters=None,
              unroll=1, staggered=False):
    runner = _get_runner(reps, loop_iters, unroll, staggered)
    return _Res(runner(in_maps, device_inputs=device_inputs))


def kernel(node_embeddings, W1, b1, W2, b2, idx_prio, idx_rest,
           neg_idx_prio, neg_idx_rest, num_views=2, num_layers=3):
    in_maps = make_in_maps(node_embeddings, W1, b1, W2, b2, idx_prio,
                           idx_rest, neg_idx_prio, neg_idx_rest)
    res = run_on_hw(in_maps)
    _CACHE["last_results"] = res
    total = 0.0
    for c in range(NCORES):
        lv = np.asarray(res.results[c]["out"], dtype=np.float64)
        for si, sl in enumerate(SLOTS):
            total += lv[0:sl[2], si].sum()
    return np.float32(total / COUNT)
